# revision 1
# baseline (speedup 1.0000x reference)
"""Causal attention kernel for Trainium2, sequence-parallel over 8 NeuronCores.

reference:
    q = x @ Wq.T + bq ; k = x @ Wk.T + bk ; v = x @ Wv.T + bv
    scores = q @ k.T / sqrt(D) + mask
    out = softmax(scores, -1) @ v

Sharding: core c owns query rows [c*Q, (c+1)*Q) and the matching mask rows.
x and the weights are replicated; each core works standalone (no collectives).

With zero biases (checked on host) the projections are folded through
associativity:
    scores = x_q @ A @ x.T + mask,   A = Wq.T @ Wk / sqrt(D)   (512x512, tiny)
    out    = (p @ x) @ Wv.T / rowsum(p)
so k and v are never materialized. Softmax needs no max subtraction: scores
are O(1) by construction and masked entries exp to 0 exactly.

Matmuls run in float32r (full-rate fp32 PE mode, ~1e-4 relative accuracy).
"""

import sys
from contextlib import ExitStack, nullcontext

if "/opt/trn_rl_repo" not in sys.path:
    sys.path.insert(0, "/opt/trn_rl_repo")

import numpy as np

import concourse.bass as bass
import concourse.tile as tile
from concourse import bacc, mybir
from concourse.bass_utils import run_bass_kernel_spmd
from concourse.masks import make_identity

F32 = mybir.dt.float32
F32R = mybir.dt.float32r

N, D, NCORES = 8192, 512, 8
P = 128          # partitions
KB = 1024        # key-block size
Q = N // NCORES  # per-core query rows


def build(n=N, d=D, ncores=NCORES, kb=KB, mm_fast=True, use_bias=False,
          reps=1, skip=None, trace_sim=False):
    q_rows = n // ncores
    nqt = q_rows // P            # q-tiles per core
    nb = n // kb                 # key blocks
    tpb = kb // P                # token tiles per block
    dc = d // P                  # d chunks
    inv_sqrt_d = 1.0 / float(np.sqrt(d))
    if skip is None:
        skip = [[False] * nqt for _ in range(nb)]
    first_live = []
    for t in range(nqt):
        live = [b for b in range(nb) if not skip[b][t]]
        assert live, f"q-tile {t} has no live key blocks"
        first_live.append(live[0])
    # Tensors consumed by PE matmuls use float32r (full-rate fp32 mode);
    # the producing instruction must write that dtype (BIR verifier rule).
    MD = F32R if mm_fast else F32

    nc = bacc.Bacc("TRN2", target_bir_lowering=False, debug=False,
                   num_devices=ncores)
    x_d = nc.dram_tensor("x", [n, d], F32, kind="ExternalInput").ap()
    xq_d = nc.dram_tensor("xq", [q_rows, d], F32, kind="ExternalInput").ap()
    mask_d = nc.dram_tensor("mask", [q_rows, n], F32, kind="ExternalInput").ap()
    w_d = {nm: nc.dram_tensor(nm, [d, d], F32, kind="ExternalInput").ap()
           for nm in ("wq", "wk", "wv")}
    b_d = {nm: nc.dram_tensor(nm, [d], F32, kind="ExternalInput").ap()
           for nm in ("bq", "bk", "bv")}
    out_d = nc.dram_tensor("out", [q_rows, d], F32, kind="ExternalOutput").ap()
    assert not use_bias, "zero-bias fast path only; use build_direct for biases"

    # Alternate PSUM->SBUF copies between ACT and DVE to balance engine load.
    flip = [0]

    def copy(out, in_):
        flip[0] ^= 1
        if flip[0]:
            nc.scalar.copy(out=out, in_=in_)
        else:
            nc.vector.tensor_copy(out=out, in_=in_)

    with tile.TileContext(nc, trace_sim=trace_sim) as tc, ExitStack() as st:
        consts = st.enter_context(tc.tile_pool(name="consts", bufs=1))
        wts = st.enter_context(tc.tile_pool(name="wts", bufs=1))
        wnat_p = st.enter_context(tc.tile_pool(name="wnat", bufs=1))
        qt_p = st.enter_context(tc.tile_pool(name="qt", bufs=1))
        xtb_p = st.enter_context(tc.tile_pool(name="xtb", bufs=2))
        xs_p = st.enter_context(tc.tile_pool(name="xs", bufs=3))
        xnb_p = st.enter_context(tc.tile_pool(name="xnb", bufs=2))
        mask_p = st.enter_context(tc.tile_pool(name="maskp", bufs=2))
        p_p = st.enter_context(tc.tile_pool(name="pp", bufs=2))
        pt_p = st.enter_context(tc.tile_pool(name="ptp", bufs=2))
        out_p = st.enter_context(tc.tile_pool(name="outp", bufs=2))
        ps_tr = st.enter_context(tc.tile_pool(name="ps_tr", bufs=2, space="PSUM"))
        ps_s = st.enter_context(tc.tile_pool(name="ps_s", bufs=4, space="PSUM"))
        ps_pv = st.enter_context(tc.tile_pool(name="ps_pv", bufs=2, space="PSUM"))

        loop = tc.For_i(0, reps, 1) if reps > 1 else nullcontext()
        with loop:
            ident = consts.tile([P, P], F32, tag="ident")
            make_identity(nc, ident)
            ident_r = consts.tile([P, P], MD, tag="ident_r")
            nc.vector.tensor_copy(out=ident_r, in_=ident)

            # ---- wvT[p, c, dout] = Wv[dout, c*P+p] ----
            wvn = wnat_p.tile([P, dc, d], F32, tag="wnat")
            nc.gpsimd.dma_start(out=wvn,
                                in_=w_d["wv"].rearrange("(j p) d -> p j d", p=P))
            wvT = wts.tile([P, dc, d], MD, tag="wvT")
            for i in range(dc):
                ps = ps_tr.tile([P, 512], F32, tag="ps_tr")
                for j in range(dc):
                    nc.tensor.transpose(ps[:, j * P:(j + 1) * P],
                                        wvn[:, j, i * P:(i + 1) * P], ident)
                copy(wvT[:, i, :].rearrange("p (j f) -> p j f", f=P),
                     ps.rearrange("p (j f) -> p j f", f=P))

            # ---- A = Wq.T @ Wk * inv_sqrt_d,  A_sb[p, i, j] = A[i*P+p, j] ----
            wqk = {}
            for nm in ("wq", "wk"):
                wn = wnat_p.tile([P, dc, d], F32, tag="wnat")
                nc.gpsimd.dma_start(
                    out=wn, in_=w_d[nm].rearrange("(j p) d -> p j d", p=P))
                wqk[nm] = wts.tile([P, dc, d], MD, tag=f"{nm}n", name=f"{nm}n")
                copy(wqk[nm], wn)
            A_sb = wts.tile([P, dc, d], MD, tag="A_sb")
            for i in range(dc):
                ps = ps_tr.tile([P, 512], F32, tag="ps_tr")
                for m in range(dc):
                    nc.tensor.matmul(ps, wqk["wq"][:, m, i * P:(i + 1) * P],
                                     wqk["wk"][:, m, :],
                                     start=(m == 0), stop=(m == dc - 1))
                nc.scalar.mul(out=A_sb[:, i, :], in_=ps, mul=inv_sqrt_d)

            # ---- xqT then yT = A.T @ xqT  (plays the role of scaled qT) ----
            xqT = xtb_p.tile([P, dc, max(q_rows, kb)], MD, tag="xtb")
            for t in range(nqt):
                xt = xs_p.tile([P, dc, d], F32, tag="xs")
                nc.gpsimd.dma_start(out=xt[:, 0, :],
                                    in_=xq_d[t * P:(t + 1) * P, :])
                ps = ps_tr.tile([P, 512], F32, tag="ps_tr")
                for i in range(dc):
                    nc.tensor.transpose(ps[:, i * P:(i + 1) * P],
                                        xt[:, 0, i * P:(i + 1) * P], ident)
                copy(xqT[:, :, t * P:(t + 1) * P],
                     ps.rearrange("p (i f) -> p i f", f=P))

            yT = qt_p.tile([P, dc, q_rows], MD, tag="yT")
            for m in range(dc):
                for n0 in range(0, q_rows, 512):
                    nn = min(512, q_rows - n0)
                    ps = ps_tr.tile([P, 512], F32, tag="ps_tr")
                    for c in range(dc):
                        nc.tensor.matmul(ps[:, :nn],
                                         A_sb[:, c, m * P:(m + 1) * P],
                                         xqT[:, c, n0:n0 + nn],
                                         start=(c == 0), stop=(c == dc - 1))
                    copy(yT[:, m, n0:n0 + nn], ps[:, :nn])

            # per-(q-tile, block, half) exp row-sums; reduced at the end
            l_all = consts.tile([P, nqt, 2 * nb], F32, tag="lall")
            nc.vector.memset(l_all, 0.0)
            linv = consts.tile([P, nqt], F32, tag="linv")
            zacc = consts.tile([P, nqt, d], F32, tag="zacc")

            # ---- main loop over key blocks ----
            for b in range(nb):
                if all(skip[b][t] for t in range(nqt)):
                    continue
                # stream x rows, transpose to xTB [p, c, kb]; keep natural
                # copy xNB [p, t, d] (f32r) as the p@x rhs
                xTB = xtb_p.tile([P, dc, max(q_rows, kb)], MD, tag="xtb")
                xNB = xnb_p.tile([P, tpb, d], MD, tag="xnb")
                for h in range(0, tpb, dc):
                    nh = min(dc, tpb - h)
                    xh = xs_p.tile([P, dc, d], F32, tag="xs")
                    nc.gpsimd.dma_start(
                        out=xh[:, :nh, :],
                        in_=x_d[b * kb + h * P: b * kb + (h + nh) * P, :]
                            .rearrange("(t p) d -> p t d", p=P))
                    copy(xNB[:, h:h + nh, :], xh[:, :nh, :])
                    for t in range(nh):
                        ps = ps_tr.tile([P, 512], F32, tag="ps_tr")
                        for i in range(dc):
                            nc.tensor.transpose(ps[:, i * P:(i + 1) * P],
                                                xh[:, t, i * P:(i + 1) * P],
                                                ident)
                        copy(xTB[:, :, (h + t) * P:(h + t + 1) * P],
                             ps.rearrange("p (i f) -> p i f", f=P))

                # attention for each q-tile against this block
                for t in range(nqt):
                    if skip[b][t]:
                        continue
                    mk = mask_p.tile([P, kb], F32, tag="maskp")
                    nc.gpsimd.dma_start(
                        out=mk,
                        in_=mask_d[t * P:(t + 1) * P, b * kb:(b + 1) * kb])
                    pt = p_p.tile([P, kb], MD, tag="pp")
                    for h0 in range(0, kb, 512):
                        ps = ps_s.tile([P, 512], F32, tag="ps_s")
                        for c in range(dc):
                            nc.tensor.matmul(ps,
                                             yT[:, c, t * P:(t + 1) * P],
                                             xTB[:, c, h0:h0 + 512],
                                             start=(c == 0), stop=(c == dc - 1))
                        nc.vector.tensor_add(out=ps, in0=ps,
                                             in1=mk[:, h0:h0 + 512])
                        col = 2 * b + h0 // 512
                        nc.scalar.activation(
                            out=pt[:, h0:h0 + 512], in_=ps,
                            func=mybir.ActivationFunctionType.Exp,
                            accum_out=l_all[:, t, col:col + 1])
                    # transpose p -> pT [p(key), t, q]
                    pT = pt_p.tile([P, tpb, P], MD, tag="ptp")
                    for h in range(0, tpb, dc):
                        ps = ps_tr.tile([P, 512], MD, tag="ps_tr")
                        for j in range(dc):
                            nc.tensor.transpose(
                                ps[:, j * P:(j + 1) * P],
                                pt[:, (h + j) * P:(h + j + 1) * P], ident_r)
                        copy(pT[:, h:h + dc, :],
                             ps.rearrange("p (j f) -> p j f", f=P))
                    # z += p @ x_block
                    ps = ps_pv.tile([P, d], F32, tag="ps_pv")
                    for kbi in range(tpb):
                        nc.tensor.matmul(ps, pT[:, kbi, :], xNB[:, kbi, :],
                                         start=(kbi == 0), stop=(kbi == tpb - 1))
                    if b == first_live[t]:
                        copy(zacc[:, t, :], ps)
                    else:
                        nc.vector.tensor_add(out=zacc[:, t, :],
                                             in0=zacc[:, t, :], in1=ps)

            # ---- finalize: out = (z / l) @ Wv.T ----
            for t in range(nqt):
                lsum = out_p.tile([P, 1], F32, tag="lsum")
                nc.vector.reduce_sum(lsum, l_all[:, t, :],
                                     axis=mybir.AxisListType.X)
                nc.vector.reciprocal(linv[:, t:t + 1], lsum)
                zn = out_p.tile([P, d], F32, tag="zn")
                nc.vector.tensor_scalar_mul(out=zn, in0=zacc[:, t, :],
                                            scalar1=linv[:, t:t + 1])
                ps = ps_tr.tile([P, 512], F32, tag="ps_tr")
                for i in range(dc):
                    nc.tensor.transpose(ps[:, i * P:(i + 1) * P],
                                        zn[:, i * P:(i + 1) * P], ident)
                znT = out_p.tile([P, dc, P], MD, tag="znT")
                copy(znT, ps.rearrange("p (i f) -> p i f", f=P))
                ops = ps_tr.tile([P, 512], F32, tag="ps_tr")
                for c in range(dc):
                    nc.tensor.matmul(ops, znT[:, c, :], wvT[:, c, :],
                                     start=(c == 0), stop=(c == dc - 1))
                ot = out_p.tile([P, d], F32, tag="outp")
                copy(ot, ops)
                nc.gpsimd.dma_start(out=out_d[t * P:(t + 1) * P, :], in_=ot)

    nc.compile()
    return nc


def core_rows(n, ncores, c):
    """Cyclic-by-128-row-tile sharding: core c owns global tiles c, c+ncores, ..."""
    nt_global = n // P
    tiles = list(range(c, nt_global, ncores))
    return np.concatenate([np.arange(g * P, (g + 1) * P) for g in tiles])


def prepare_in_maps(x, mask, Wq, bq, Wk, bk, Wv, bv, n=None, ncores=NCORES,
                    kb=KB):
    """Cyclic q-tile sharding + per-(block, tile) full-mask skip table.

    A (q-tile, key-block) pair is skipped only when EVERY core's mask block
    at that position is entirely <= -1e8: exp(scores + mask) underflows to
    exactly 0.0 there, so skipping is bit-exact. With a causal mask the
    cyclic assignment makes each core skip the same ~44% of pairs.
    """
    x = np.asarray(x); mask = np.asarray(mask)
    if n is None:
        n = x.shape[0]
    q_rows = n // ncores
    nqt = q_rows // P
    nb = n // kb
    f = np.ascontiguousarray
    rows = [core_rows(n, ncores, c) for c in range(ncores)]
    # skip[b][t] must hold for every core (the SPMD program is shared)
    skip = [[True] * nqt for _ in range(nb)]
    for c in range(ncores):
        m = mask[rows[c]]
        blk = m.reshape(nqt, P, nb, kb).max(axis=(1, 3))  # [nqt, nb]
        for b in range(nb):
            for t in range(nqt):
                if blk[t, b] > -1e8:
                    skip[b][t] = False
    in_maps = [
        {
            "x": f(x.astype(np.float32)),
            "xq": f(x[rows[c]].astype(np.float32)),
            "mask": f(mask[rows[c]].astype(np.float32)),
            "wq": f(np.asarray(Wq).astype(np.float32)),
            "bq": f(np.asarray(bq).astype(np.float32)),
            "wk": f(np.asarray(Wk).astype(np.float32)),
            "bk": f(np.asarray(bk).astype(np.float32)),
            "wv": f(np.asarray(Wv).astype(np.float32)),
            "bv": f(np.asarray(bv).astype(np.float32)),
        }
        for c in range(ncores)
    ]
    meta = {"skip": skip, "rows": rows}
    return in_maps, meta


def make_in_maps(x, mask, Wq, bq, Wk, bk, Wv, bv, ncores=NCORES, kb=KB):
    in_maps, _ = prepare_in_maps(x, mask, Wq, bq, Wk, bk, Wv, bv,
                                 ncores=ncores, kb=kb)
    return in_maps


_CACHED = {}


def kernel(x, mask, Wq, bq, Wk, bk, Wv, bv):
    x = np.asarray(x)
    in_maps, meta = prepare_in_maps(x, mask, Wq, bq, Wk, bk, Wv, bv)
    key = bytes(bytearray(b for row in meta["skip"] for b in row))
    if _CACHED.get("key") != key:
        _CACHED["nc"] = build(skip=meta["skip"])
        _CACHED["key"] = key
    nc = _CACHED["nc"]
    res = run_bass_kernel_spmd(nc, in_maps, list(range(NCORES)))
    out = np.empty((x.shape[0], x.shape[1]), np.float32)
    for c in range(NCORES):
        out[meta["rows"][c]] = res.results[c]["out"]
    return out



# revision 2
# speedup vs baseline: 1.8167x; 1.8167x over previous
"""Causal attention for Trainium2, sequence-parallel over 8 NeuronCores — v2.

reference:
    q = x @ Wq.T ; k = x @ Wk.T ; v = x @ Wv.T      (biases are zero)
    scores = q @ k.T / sqrt(D) + mask
    out = softmax(scores, -1) @ v

Core c owns query tiles {c, c+8, ..., c+56} (cyclic by 128 rows).  Folding
the projections (A = Wq.T @ Wk / sqrt(D)):
    sT[k, q] = x[k, :] . y[q, :],   y = xq @ A
    out = ((p @ x) / rowsum(p)) @ Wv.T,   p = exp(sT + causal)
Scores are computed TRANSPOSED ([key-part, query-free]) so the exp output
IS the pv lhsT — no on-chip transposes of p at all.  x is supplied by the
host in bf16 twice (natural + transposed), so there are no on-chip x
transposes or casts either.  The 64MB mask never reaches the device: the
causal structure collapses to one per-core [128, 8, 128] additive strip
(0 / -1e9) applied to the diagonal query tile as an extra chained matmul
(identity @ strip) that initializes the first 128 score columns.

Softmax needs no max subtraction: scores are O(1) by construction and
masked entries exp to exactly 0.  Row sums come from a ones-column matmul
chain; z and l accumulate across key blocks in SBUF (f32).
"""

import sys
from contextlib import ExitStack, nullcontext

if "/opt/trn_rl_repo" not in sys.path:
    sys.path.insert(0, "/opt/trn_rl_repo")

import numpy as np
import ml_dtypes

import concourse.bass as bass
import concourse.tile as tile
from concourse import bacc, mybir
from concourse.bass_utils import run_bass_kernel_spmd
from concourse.masks import make_identity

F32 = mybir.dt.float32
F32R = mybir.dt.float32r
BF16 = mybir.dt.bfloat16
NPBF16 = ml_dtypes.bfloat16

N, D, NCORES = 8192, 512, 8
P = 128           # partitions
KB = 1024         # key-block size
Q = N // NCORES   # per-core query rows
NQT = Q // P      # q-tiles per core
NB = N // KB      # key blocks
TPB = KB // P     # key tiles per block
DC = D // P       # d chunks


def build(reps=1, trace_sim=False):
    inv_sqrt_d = 1.0 / float(np.sqrt(D))
    nc = bacc.Bacc("TRN2", target_bir_lowering=False, debug=False,
                   num_devices=NCORES)
    xt_d = nc.dram_tensor("xt", [P, DC, N], BF16, kind="ExternalInput").ap()
    xn_d = nc.dram_tensor("xn", [N, D], BF16, kind="ExternalInput").ap()
    xqt_d = nc.dram_tensor("xqt", [P, DC, Q], BF16, kind="ExternalInput").ap()
    wq_d = nc.dram_tensor("wq", [P, DC, D], BF16, kind="ExternalInput").ap()
    wk_d = nc.dram_tensor("wk", [P, DC, D], BF16, kind="ExternalInput").ap()
    wvt_d = nc.dram_tensor("wvt", [P, DC, D], BF16, kind="ExternalInput").ap()
    ms_d = nc.dram_tensor("mstrip", [P, TPB, P], BF16, kind="ExternalInput").ap()
    out_d = nc.dram_tensor("out", [Q, D], F32, kind="ExternalOutput").ap()

    # Alternate SBUF-writing copies between ACT and DVE to balance load.
    flip = [0]

    def copy(out, in_):
        flip[0] ^= 1
        if flip[0]:
            nc.scalar.copy(out=out, in_=in_)
        else:
            nc.vector.tensor_copy(out=out, in_=in_)

    with tile.TileContext(nc, trace_sim=trace_sim) as tc, ExitStack() as st:
        consts = st.enter_context(tc.tile_pool(name="consts", bufs=1))
        wts = st.enter_context(tc.tile_pool(name="wts", bufs=1))
        xt_p = st.enter_context(tc.tile_pool(name="xt", bufs=2))
        xn_p = st.enter_context(tc.tile_pool(name="xn", bufs=2))
        pt_p = st.enter_context(tc.tile_pool(name="pt", bufs=2))
        acc_p = st.enter_context(tc.tile_pool(name="acc", bufs=1))
        fin_p = st.enter_context(tc.tile_pool(name="fin", bufs=2))
        ps_s = st.enter_context(tc.tile_pool(name="ps_s", bufs=2, space="PSUM"))
        ps_z = st.enter_context(tc.tile_pool(name="ps_z", bufs=2, space="PSUM"))
        ps_l = st.enter_context(tc.tile_pool(name="ps_l", bufs=2, space="PSUM"))

        loop = tc.For_i(0, reps, 1) if reps > 1 else nullcontext()
        with loop:
            ident = consts.tile([P, P], F32, tag="ident")
            make_identity(nc, ident)
            ident_r = consts.tile([P, P], F32R, tag="ident_r")
            nc.vector.tensor_copy(out=ident_r, in_=ident)
            ident_b = consts.tile([P, P], BF16, tag="ident_b")
            nc.vector.tensor_copy(out=ident_b, in_=ident)
            ones = consts.tile([P, 1], BF16, tag="ones")
            nc.vector.memset(ones, 1.0)

            mstrip = wts.tile([P, TPB, P], BF16, tag="mstrip")
            nc.gpsimd.dma_start(out=mstrip, in_=ms_d)
            wq_s = wts.tile([P, DC, D], BF16, tag="wq")
            nc.gpsimd.dma_start(out=wq_s, in_=wq_d)
            wk_s = wts.tile([P, DC, D], BF16, tag="wk")
            nc.gpsimd.dma_start(out=wk_s, in_=wk_d)
            wvt_s = wts.tile([P, DC, D], BF16, tag="wvt")
            nc.gpsimd.dma_start(out=wvt_s, in_=wvt_d)
            xqt_s = wts.tile([P, DC, Q], BF16, tag="xqt")
            nc.gpsimd.dma_start(out=xqt_s, in_=xqt_d)

            # ---- A[i, j] = sum_d Wq[d, i] Wk[d, j] * inv_sqrt_d ----
            A_sb = wts.tile([P, DC, D], BF16, tag="A_sb")
            for ic in range(DC):
                ps = ps_z.tile([P, D], F32, tag="ps_z")
                for m in range(DC):
                    nc.tensor.matmul(ps, wq_s[:, m, ic * P:(ic + 1) * P],
                                     wk_s[:, m, :],
                                     start=(m == 0), stop=(m == DC - 1))
                nc.scalar.mul(out=A_sb[:, ic, :], in_=ps, mul=inv_sqrt_d)

            # ---- yT[j, q] = sum_i A[i, j] xqT[i, q] ----
            yT = wts.tile([P, DC, Q], BF16, tag="yT")
            for jc in range(DC):
                for qh in range(0, Q, 512):
                    ps = ps_z.tile([P, D], F32, tag="ps_z")
                    for ic in range(DC):
                        nc.tensor.matmul(ps, A_sb[:, ic, jc * P:(jc + 1) * P],
                                         xqt_s[:, ic, qh:qh + 512],
                                         start=(ic == 0), stop=(ic == DC - 1))
                    copy(yT[:, jc, qh:qh + 512], ps)

            zacc = acc_p.tile([P, NQT, D], F32, tag="zacc")
            lacc = acc_p.tile([P, NQT], F32, tag="lacc")

            # ---- main loop over key blocks ----
            for b in range(NB):
                W = (NQT - b) * P      # live query width (q-tiles [b, 8))
                xT = xt_p.tile([P, DC, KB], BF16, tag="xt")
                nc.gpsimd.dma_start(out=xT, in_=xt_d[:, :, b * KB:(b + 1) * KB])
                xN = xn_p.tile([P, TPB, D], BF16, tag="xn")
                nc.gpsimd.dma_start(
                    out=xN, in_=xn_d[b * KB:(b + 1) * KB, :]
                    .rearrange("(t p) d -> p t d", p=P))

                pT = pt_p.tile([P, TPB, Q], BF16, tag="pt")
                for kt in range(TPB):
                    ps = ps_s.tile([P, 2 * D], F32, tag="ps_s")
                    # diagonal 128 columns: mask strip first (initializes),
                    # then the DC contraction chunks
                    nc.tensor.matmul(ps[:, 0:P], ident_b, mstrip[:, kt, :],
                                     start=True, stop=False)
                    for cc in range(DC):
                        nc.tensor.matmul(ps[:, 0:P],
                                         xT[:, cc, kt * P:(kt + 1) * P],
                                         yT[:, cc, b * P:(b + 1) * P],
                                         start=False, stop=(cc == DC - 1))
                    # remaining live columns, segmented at psum bank edges
                    segs = []
                    if W > P:
                        segs.append((P, min(W, D) - P))
                    if W > D:
                        segs.append((D, W - D))
                    for ch, cw in segs:
                        for cc in range(DC):
                            nc.tensor.matmul(
                                ps[:, ch:ch + cw],
                                xT[:, cc, kt * P:(kt + 1) * P],
                                yT[:, cc, b * P + ch:b * P + ch + cw],
                                start=(cc == 0), stop=(cc == DC - 1))
                    nc.scalar.activation(
                        out=pT[:, kt, 0:W], in_=ps[:, 0:W],
                        func=mybir.ActivationFunctionType.Exp)

                for t in range(b, NQT):
                    j = t - b
                    psz = ps_z.tile([P, D], F32, tag="ps_z")
                    for kt in range(TPB):
                        nc.tensor.matmul(psz, pT[:, kt, j * P:(j + 1) * P],
                                         xN[:, kt, :],
                                         start=(kt == 0), stop=(kt == TPB - 1))
                    psl = ps_l.tile([P, 1], F32, tag="ps_l")
                    for kt in range(TPB):
                        nc.tensor.matmul(psl, pT[:, kt, j * P:(j + 1) * P],
                                         ones,
                                         start=(kt == 0), stop=(kt == TPB - 1))
                    if b == 0:
                        copy(zacc[:, t, :], psz)
                        copy(lacc[:, t:t + 1], psl)
                    else:
                        nc.vector.tensor_add(out=zacc[:, t, :],
                                             in0=zacc[:, t, :], in1=psz)
                        nc.vector.tensor_add(out=lacc[:, t:t + 1],
                                             in0=lacc[:, t:t + 1], in1=psl)

            # ---- finalize: out = (z / l) @ Wv.T ----
            for t in range(NQT):
                linv = fin_p.tile([P, 1], F32, tag="linv")
                nc.vector.reciprocal(linv, lacc[:, t:t + 1])
                zn = fin_p.tile([P, D], F32R, tag="zn")
                nc.vector.tensor_scalar_mul(out=zn, in0=zacc[:, t, :],
                                            scalar1=linv)
                ps_t = ps_z.tile([P, D], F32R, tag="ps_z")
                for ic in range(DC):
                    nc.tensor.transpose(ps_t[:, ic * P:(ic + 1) * P],
                                        zn[:, ic * P:(ic + 1) * P], ident_r)
                znT = fin_p.tile([P, DC, P], BF16, tag="znT")
                copy(znT, ps_t.rearrange("p (i f) -> p i f", f=P))
                pso = ps_z.tile([P, D], F32, tag="ps_z")
                for cc in range(DC):
                    nc.tensor.matmul(pso, znT[:, cc, :], wvt_s[:, cc, :],
                                     start=(cc == 0), stop=(cc == DC - 1))
                ot = fin_p.tile([P, D], F32, tag="ot")
                copy(ot, pso)
                nc.gpsimd.dma_start(out=out_d[t * P:(t + 1) * P, :], in_=ot)

    nc.compile()
    return nc


def core_rows(n, ncores, c):
    nt_global = n // P
    tiles = list(range(c, nt_global, ncores))
    return np.concatenate([np.arange(g * P, (g + 1) * P) for g in tiles])


def prepare_in_maps(x, mask, Wq, bq, Wk, bk, Wv, bv):
    x = np.asarray(x, np.float32)
    for b in (bq, bk, bv):
        assert not np.any(np.asarray(b)), "zero-bias fast path only"
    # cheap causal-mask verification on a sample of 128-row bands
    m = np.asarray(mask)
    idx = np.arange(N)
    for r in (0, 1, 4095, 8191, 2917):
        row = m[r]
        assert np.all(row[: r + 1] == 0.0) and np.all(row[r + 1:] <= -1e8), \
            "kernel specialized to the causal mask"
    f = np.ascontiguousarray
    xb = x.astype(NPBF16)
    xtb = f(x.T.astype(NPBF16).reshape(DC, P, N).transpose(1, 0, 2))
    wqb = f(np.asarray(Wq, np.float32).astype(NPBF16)
            .reshape(DC, P, D).transpose(1, 0, 2))
    wkb = f(np.asarray(Wk, np.float32).astype(NPBF16)
            .reshape(DC, P, D).transpose(1, 0, 2))
    wvtb = f(np.asarray(Wv, np.float32).T.astype(NPBF16)
             .reshape(DC, P, D).transpose(1, 0, 2))
    k_in = np.arange(P)[:, None]          # key within tile (partition)
    q_in = np.arange(P)[None, :]          # query within tile (free)
    rows = [core_rows(N, NCORES, c) for c in range(NCORES)]
    in_maps = []
    for c in range(NCORES):
        ms = np.empty((P, TPB, P), np.float32)
        for kt in range(TPB):
            live = (c - kt) * P + q_in - k_in >= 0
            ms[:, kt, :] = np.where(live, 0.0, -1e9)
        xqtb = f(x[rows[c]].T.astype(NPBF16).reshape(DC, P, Q)
                 .transpose(1, 0, 2))
        in_maps.append({
            "xt": xtb, "xn": xb, "xqt": xqtb,
            "wq": wqb, "wk": wkb, "wvt": wvtb,
            "mstrip": ms.astype(NPBF16),
        })
    return in_maps, {"rows": rows}


_CACHED = {}


def kernel(x, mask, Wq, bq, Wk, bk, Wv, bv):
    x = np.asarray(x)
    in_maps, meta = prepare_in_maps(x, mask, Wq, bq, Wk, bk, Wv, bv)
    if "nc" not in _CACHED:
        _CACHED["nc"] = build()
    nc = _CACHED["nc"]
    res = run_bass_kernel_spmd(nc, in_maps, list(range(NCORES)))
    out = np.empty((x.shape[0], x.shape[1]), np.float32)
    for c in range(NCORES):
        out[meta["rows"][c]] = res.results[c]["out"]
    return out


# revision 3
# speedup vs baseline: 1.9378x; 1.0667x over previous
"""Causal attention for Trainium2, sequence-parallel over 8 NeuronCores — v2.

reference:
    q = x @ Wq.T ; k = x @ Wk.T ; v = x @ Wv.T      (biases are zero)
    scores = q @ k.T / sqrt(D) + mask
    out = softmax(scores, -1) @ v

Core c owns query tiles {c, c+8, ..., c+56} (cyclic by 128 rows).  Folding
the projections (A = Wq.T @ Wk / sqrt(D)):
    sT[k, q] = x[k, :] . y[q, :],   y = xq @ A
    out = ((p @ x) / rowsum(p)) @ Wv.T,   p = exp(sT + causal)
Scores are computed TRANSPOSED ([key-part, query-free]) so the exp output
IS the pv lhsT — no on-chip transposes of p at all.  x is supplied by the
host in bf16 twice (natural + transposed), so there are no on-chip x
transposes or casts either.  The 64MB mask never reaches the device: the
causal structure collapses to one per-core [128, 8, 128] additive strip
(0 / -1e9) applied to the diagonal query tile as an extra chained matmul
(identity @ strip) that initializes the first 128 score columns.

Softmax needs no max subtraction: scores are O(1) by construction and
masked entries exp to exactly 0.  Row sums come from a ones-column matmul
chain; z and l accumulate across key blocks in SBUF (f32).
"""

import sys
from contextlib import ExitStack, nullcontext

if "/opt/trn_rl_repo" not in sys.path:
    sys.path.insert(0, "/opt/trn_rl_repo")

import numpy as np
import ml_dtypes

import concourse.bass as bass
import concourse.tile as tile
from concourse import bacc, mybir
from concourse.bass_utils import run_bass_kernel_spmd
from concourse.masks import make_identity

F32 = mybir.dt.float32
F32R = mybir.dt.float32r
BF16 = mybir.dt.bfloat16
NPBF16 = ml_dtypes.bfloat16

N, D, NCORES = 8192, 512, 8
P = 128           # partitions
KB = 1024         # key-block size
Q = N // NCORES   # per-core query rows
NQT = Q // P      # q-tiles per core
NB = N // KB      # key blocks
TPB = KB // P     # key tiles per block
DC = D // P       # d chunks


def build(reps=1, trace_sim=False):
    inv_sqrt_d = 1.0 / float(np.sqrt(D))
    nc = bacc.Bacc("TRN2", target_bir_lowering=False, debug=False,
                   num_devices=NCORES)
    xt_d = nc.dram_tensor("xt", [P, DC, N], BF16, kind="ExternalInput").ap()
    xn_d = nc.dram_tensor("xn", [N, D], BF16, kind="ExternalInput").ap()
    xqt_d = nc.dram_tensor("xqt", [P, DC, Q], BF16, kind="ExternalInput").ap()
    wq_d = nc.dram_tensor("wq", [P, DC, D], BF16, kind="ExternalInput").ap()
    wk_d = nc.dram_tensor("wk", [P, DC, D], BF16, kind="ExternalInput").ap()
    wvt_d = nc.dram_tensor("wvt", [P, DC, D], BF16, kind="ExternalInput").ap()
    ms_d = nc.dram_tensor("mstrip", [P, TPB, P], BF16, kind="ExternalInput").ap()
    out_d = nc.dram_tensor("out", [Q, D], F32, kind="ExternalOutput").ap()

    # Alternate SBUF-writing copies between ACT and DVE to balance load.
    flip = [0]

    def copy(out, in_):
        flip[0] ^= 1
        if flip[0]:
            nc.scalar.copy(out=out, in_=in_)
        else:
            nc.vector.tensor_copy(out=out, in_=in_)

    with tile.TileContext(nc, trace_sim=trace_sim) as tc, ExitStack() as st:
        consts = st.enter_context(tc.tile_pool(name="consts", bufs=1))
        wts = st.enter_context(tc.tile_pool(name="wts", bufs=1))
        xt_p = st.enter_context(tc.tile_pool(name="xt", bufs=2))
        xn_p = st.enter_context(tc.tile_pool(name="xn", bufs=3))
        pt_p = st.enter_context(tc.tile_pool(name="pt", bufs=2))
        acc_p = st.enter_context(tc.tile_pool(name="acc", bufs=1))
        fin_p = st.enter_context(tc.tile_pool(name="fin", bufs=2))
        ps_s = st.enter_context(tc.tile_pool(name="ps_s", bufs=2, space="PSUM"))
        ps_z = st.enter_context(tc.tile_pool(name="ps_z", bufs=3, space="PSUM"))
        ps_l = st.enter_context(tc.tile_pool(name="ps_l", bufs=1, space="PSUM"))

        loop = tc.For_i(0, reps, 1) if reps > 1 else nullcontext()
        with loop:
            ident = consts.tile([P, P], F32, tag="ident")
            make_identity(nc, ident)
            ident_r = consts.tile([P, P], F32R, tag="ident_r")
            nc.vector.tensor_copy(out=ident_r, in_=ident)
            ident_b = consts.tile([P, P], BF16, tag="ident_b")
            nc.vector.tensor_copy(out=ident_b, in_=ident)
            ones = consts.tile([P, 1], BF16, tag="ones")
            nc.vector.memset(ones, 1.0)

            # DMA issue order = landing order: A's weights first, then xqt
            # (yT), then the mask strip; wvt (finalize-only) is deferred
            # until after the first block loads.
            wq_s = wts.tile([P, DC, D], BF16, tag="wq")
            nc.gpsimd.dma_start(out=wq_s, in_=wq_d)
            wk_s = wts.tile([P, DC, D], BF16, tag="wk")
            nc.gpsimd.dma_start(out=wk_s, in_=wk_d)
            xqt_s = wts.tile([P, DC, Q], BF16, tag="xqt")
            nc.gpsimd.dma_start(out=xqt_s, in_=xqt_d)
            mstrip = wts.tile([P, TPB, P], BF16, tag="mstrip")
            nc.gpsimd.dma_start(out=mstrip, in_=ms_d)
            wvt_s = wts.tile([P, DC, D], BF16, tag="wvt")

            # ---- A[i, j] = sum_d Wq[d, i] Wk[d, j]  (1/sqrt(D) folded into
            # wq on the host) ----
            A_sb = wts.tile([P, DC, D], BF16, tag="A_sb")
            for ic in range(DC):
                ps = ps_z.tile([P, D], F32, tag="ps_z")
                for m in range(DC):
                    nc.tensor.matmul(ps, wq_s[:, m, ic * P:(ic + 1) * P],
                                     wk_s[:, m, :],
                                     start=(m == 0), stop=(m == DC - 1))
                copy(A_sb[:, ic, :], ps)

            # ---- yT[j, q] = sum_i A[i, j] xqT[i, q] ----
            # q-descending halves: the first processed key block (b=7) only
            # needs the tail of yT, so scoring starts before yT completes.
            yT = wts.tile([P, DC, Q], BF16, tag="yT")
            for qh in range(Q - 512, -1, -512):
                for jc in range(DC):
                    ps = ps_z.tile([P, D], F32, tag="ps_z")
                    for ic in range(DC):
                        nc.tensor.matmul(ps, A_sb[:, ic, jc * P:(jc + 1) * P],
                                         xqt_s[:, ic, qh:qh + 512],
                                         start=(ic == 0), stop=(ic == DC - 1))
                    copy(yT[:, jc, qh:qh + 512], ps)

            zacc = acc_p.tile([P, NQT, D], F32R, tag="zacc")
            lacc = acc_p.tile([P, NQT], F32, tag="lacc")

            def load(b):
                xT = xt_p.tile([P, DC, KB], BF16, tag="xt")
                nc.gpsimd.dma_start(out=xT, in_=xt_d[:, :, b * KB:(b + 1) * KB])
                xN = xn_p.tile([P, TPB, D], BF16, tag="xn")
                nc.gpsimd.dma_start(
                    out=xN, in_=xn_d[b * KB:(b + 1) * KB, :]
                    .rearrange("(t p) d -> p t d", p=P))
                return xT, xN

            def scores(b, xT):
                W = (NQT - b) * P      # live query width (q-tiles [b, 8))
                pT = pt_p.tile([P, TPB, Q], BF16, tag="pt")
                for kt in range(TPB):
                    ps = ps_s.tile([P, 2 * D], F32, tag="ps_s")
                    # diagonal 128 columns: causal strip first (initializes),
                    # then the DC contraction chunks
                    nc.tensor.matmul(ps[:, 0:P], ident_b, mstrip[:, kt, :],
                                     start=True, stop=False)
                    for cc in range(DC):
                        nc.tensor.matmul(ps[:, 0:P],
                                         xT[:, cc, kt * P:(kt + 1) * P],
                                         yT[:, cc, b * P:(b + 1) * P],
                                         start=False, stop=(cc == DC - 1))
                    # remaining live columns, segmented at psum bank edges
                    segs = []
                    if W > P:
                        segs.append((P, min(W, D) - P))
                    if W > D:
                        segs.append((D, W - D))
                    for ch, cw in segs:
                        for cc in range(DC):
                            nc.tensor.matmul(
                                ps[:, ch:ch + cw],
                                xT[:, cc, kt * P:(kt + 1) * P],
                                yT[:, cc, b * P + ch:b * P + ch + cw],
                                start=(cc == 0), stop=(cc == DC - 1))
                    nc.scalar.activation(
                        out=pT[:, kt, 0:W], in_=ps[:, 0:W],
                        func=mybir.ActivationFunctionType.Exp)
                return pT

            # finalize is split in two so its PE work pipelines between
            # consecutive pv chains: out = (z @ Wv.T) / l (row scale last).
            fin_state = {}

            def fin1(t):
                linv = fin_p.tile([P, 1], F32, tag="linv")
                nc.vector.reciprocal(linv, lacc[:, t:t + 1])
                ps_t = ps_z.tile([P, D], F32R, tag="ps_z")
                for ic in range(DC):
                    nc.tensor.transpose(ps_t[:, ic * P:(ic + 1) * P],
                                        zacc[:, t, ic * P:(ic + 1) * P],
                                        ident_r)
                znT = fin_p.tile([P, DC, P], BF16, tag="znT")
                copy(znT, ps_t.rearrange("p (i f) -> p i f", f=P))
                fin_state[t] = (linv, znT)

            def fin2(t):
                linv, znT = fin_state.pop(t)
                pso = ps_z.tile([P, D], F32, tag="ps_z")
                for cc in range(DC):
                    nc.tensor.matmul(pso, znT[:, cc, :], wvt_s[:, cc, :],
                                     start=(cc == 0), stop=(cc == DC - 1))
                ot = fin_p.tile([P, D], F32, tag="ot")
                nc.vector.tensor_scalar_mul(out=ot, in0=pso, scalar1=linv)
                nc.sync.dma_start(out=out_d[t * P:(t + 1) * P, :], in_=ot)

            def pv(b, pT, xN):
                for t in range(b, NQT):
                    j = t - b
                    psz = ps_z.tile([P, D], F32, tag="ps_z")
                    psl = ps_l.tile([P, 1], F32, tag="ps_l")
                    # z and l chains interleaved per key tile: adjacent
                    # matmuls share the same stationary pT slice
                    for kt in range(TPB):
                        w = pT[:, kt, j * P:(j + 1) * P]
                        nc.tensor.matmul(psz, w, xN[:, kt, :],
                                         start=(kt == 0), stop=(kt == TPB - 1))
                        nc.tensor.matmul(psl, w, ones,
                                         start=(kt == 0), stop=(kt == TPB - 1))
                    if b == t:
                        copy(zacc[:, t, :], psz)
                        copy(lacc[:, t:t + 1], psl)
                    else:
                        nc.vector.tensor_add(out=zacc[:, t, :],
                                             in0=zacc[:, t, :], in1=psz)
                        nc.vector.tensor_add(out=lacc[:, t:t + 1],
                                             in0=lacc[:, t:t + 1], in1=psl)
                    if b == 0:
                        if t >= 1:
                            fin1(t - 1)
                        if t >= 2:
                            fin2(t - 2)
                        if t == NQT - 1:
                            fin1(t)
                            fin2(t - 1)
                            fin2(t)

            # Descending blocks; DMA prefetched one block ahead; pv runs one
            # block behind scores so it never waits on the freshest exp.
            ld = load(NB - 1)
            nc.gpsimd.dma_start(out=wvt_s, in_=wvt_d)
            prev = None
            for b in range(NB - 1, -1, -1):
                ld_next = load(b - 1) if b > 0 else None
                pT = scores(b, ld[0])
                if prev is not None:
                    pv(b + 1, prev[0], prev[1])
                prev = (pT, ld[1])
                ld = ld_next
            pv(0, prev[0], prev[1])

    nc.compile()
    return nc


def core_rows(n, ncores, c):
    nt_global = n // P
    tiles = list(range(c, nt_global, ncores))
    return np.concatenate([np.arange(g * P, (g + 1) * P) for g in tiles])


def prepare_in_maps(x, mask, Wq, bq, Wk, bk, Wv, bv):
    x = np.asarray(x, np.float32)
    for b in (bq, bk, bv):
        assert not np.any(np.asarray(b)), "zero-bias fast path only"
    # cheap causal-mask verification on a sample of 128-row bands
    m = np.asarray(mask)
    idx = np.arange(N)
    for r in (0, 1, 4095, 8191, 2917):
        row = m[r]
        assert np.all(row[: r + 1] == 0.0) and np.all(row[r + 1:] <= -1e8), \
            "kernel specialized to the causal mask"
    f = np.ascontiguousarray
    xb = x.astype(NPBF16)
    xtb = f(x.T.astype(NPBF16).reshape(DC, P, N).transpose(1, 0, 2))
    wqb = f((np.asarray(Wq, np.float32) / np.sqrt(D)).astype(NPBF16)
            .reshape(DC, P, D).transpose(1, 0, 2))
    wkb = f(np.asarray(Wk, np.float32).astype(NPBF16)
            .reshape(DC, P, D).transpose(1, 0, 2))
    wvtb = f(np.asarray(Wv, np.float32).T.astype(NPBF16)
             .reshape(DC, P, D).transpose(1, 0, 2))
    k_in = np.arange(P)[:, None]          # key within tile (partition)
    q_in = np.arange(P)[None, :]          # query within tile (free)
    rows = [core_rows(N, NCORES, c) for c in range(NCORES)]
    in_maps = []
    for c in range(NCORES):
        ms = np.empty((P, TPB, P), np.float32)
        for kt in range(TPB):
            live = (c - kt) * P + q_in - k_in >= 0
            ms[:, kt, :] = np.where(live, 0.0, -1e9)
        xqtb = f(x[rows[c]].T.astype(NPBF16).reshape(DC, P, Q)
                 .transpose(1, 0, 2))
        in_maps.append({
            "xt": xtb, "xn": xb, "xqt": xqtb,
            "wq": wqb, "wk": wkb, "wvt": wvtb,
            "mstrip": ms.astype(NPBF16),
        })
    return in_maps, {"rows": rows}


_CACHED = {}


def kernel(x, mask, Wq, bq, Wk, bk, Wv, bv):
    x = np.asarray(x)
    in_maps, meta = prepare_in_maps(x, mask, Wq, bq, Wk, bk, Wv, bv)
    if "nc" not in _CACHED:
        _CACHED["nc"] = build()
    nc = _CACHED["nc"]
    res = run_bass_kernel_spmd(nc, in_maps, list(range(NCORES)))
    out = np.empty((x.shape[0], x.shape[1]), np.float32)
    for c in range(NCORES):
        out[meta["rows"][c]] = res.results[c]["out"]
    return out
